# revision 1
# baseline (speedup 1.0000x reference)
"""BiLSTM-CRF decode kernel for Trainium2 (8 NeuronCores, batch-sharded).

Model: embedding lookup -> 2-layer BiLSTM (H=128/dir) -> linear -> CRF Viterbi.
Output: [B, T] int32 best-path tags.

Sharding: data-parallel over batch, B=128 -> 16 rows per core. Everything on
one core is laid out feature-major ([feature partitions, batch free]) so the
serial time recurrences run on full-width engine ops.
"""

import os
import numpy as np

import concourse.bass as bass
import concourse.bacc as bacc
import concourse.tile as tile
import concourse.mybir as mybir
from concourse.bass import IndirectOffsetOnAxis
from concourse import bass_utils

B, TFULL, V, D, H, K = 128, 512, 50000, 128, 128, 32
NCORES = 8
BL = B // NCORES  # 16 batch rows per core

f32 = mybir.dt.float32
bf16 = mybir.dt.bfloat16
i32 = mybir.dt.int32
u32 = mybir.dt.uint32
AF = mybir.ActivationFunctionType
ALU = mybir.AluOpType
AX = mybir.AxisListType

# torch gate order is [i, f, g, o]; we use [i, f, o, g] so the sigmoid gates
# (i, f, o) are contiguous and tanh(g) is the last chunk.
_PERM = np.r_[0:H, H:2 * H, 3 * H:4 * H, 2 * H:3 * H]

LAST_RESULTS = None  # BassKernelResults of the most recent run (for test.py)


def _f(x):
    return np.ascontiguousarray(np.asarray(x, dtype=np.float32))


def _host_consts(emb, w_ih_l0, w_hh_l0, b_l0, w_ih_l1, w_hh_l1, b_l1,
                 W_out, b_out, start_t, end_t, trans):
    """Build all per-core-identical device input arrays."""
    c = {}
    c["embt"] = _f(emb)

    for d in (0, 1):
        c[f"wx0{d}"] = _f(np.asarray(w_ih_l0)[d][_PERM].T)      # [128, 512]
        c[f"wh0{d}"] = _f(np.asarray(w_hh_l0)[d][_PERM].T)      # [128, 512]
        w1 = np.asarray(w_ih_l1)[d][_PERM]                       # [512, 256]
        c[f"wxA1{d}"] = _f(w1[:, :H].T)                          # [128, 512]
        c[f"wxB1{d}"] = _f(w1[:, H:].T)                          # [128, 512]
        c[f"wh1{d}"] = _f(np.asarray(w_hh_l1)[d][_PERM].T)       # [128, 512]

    for l, bl in ((0, b_l0), (1, b_l1)):
        bk = np.zeros((8, 128), dtype=np.float32)
        for d in (0, 1):
            bperm = np.asarray(bl)[d][_PERM]
            for ci in range(4):
                bk[d * 4 + ci, :] = bperm[ci * 128:(ci + 1) * 128]
        c[f"biasK{l}"] = _f(bk)
    ind = np.zeros((8, 128), dtype=np.float32)
    for d in (0, 1):
        for ci in range(4):
            ind[d * 4 + ci, d * 64 + ci * 16: d * 64 + (ci + 1) * 16] = 1.0
    c["chunkInd8"] = _f(ind)

    # Viterbi score columns are stored jl-major: column i' holds tag
    # pi[i'] = (i' % 4) * 8 + i' // 4 (so each pick-matmul writes a
    # contiguous 4-column block).
    pi = (np.arange(32) % 4) * 8 + np.arange(32) // 4

    WoT = _f(np.asarray(W_out).T)                                # [256, 32]
    c["woutA"] = _f(WoT[:H][:, pi])
    c["woutB"] = _f(WoT[H:][:, pi])
    c["bo1"] = _f(np.asarray(b_out)[pi][None, :])                # [1, 32]
    c["ones1"] = np.ones((1, 128), dtype=np.float32)

    km = np.arange(128)
    rep_full = (km[:, None] % 16 == km[None, :] % 16).astype(np.float32)
    for q in range(8):
        bm = rep_full.copy()
        bm[(km // 16) != q, :] = 0.0
        c[f"band{q}"] = _f(bm)

    jlv = np.arange(128) // 16                                   # [128]
    gv = 8 * np.arange(4)                                        # [4]
    trans_np = _f(trans)
    # transP[p, g*32 + i'] = trans[pi[i'], g*8 + jl(p)]
    c["transP"] = _f(trans_np[pi].T[jlv[:, None] + gv[None, :]]
                     .reshape(128, 128))
    c["iotaI"] = _f(np.tile(pi.astype(np.float32), (128, 4)))
    c["iotaF"] = _f(np.tile(pi.astype(np.float32), (BL, 1)))     # [16, 32]

    jp = np.empty(32, dtype=np.float32)
    for jl in range(8):
        for g in range(4):
            jp[jl * 4 + g] = g * 8 + jl
    c["iotaJP"] = _f(np.tile(jp, (BL, 1)))                       # [16, 32]

    c["ms0"] = _f(np.asarray(start_t)[jlv[:, None] + gv[None, :]])  # [128, 4]
    c["end128"] = _f(np.tile(np.asarray(end_t)[pi], (128, 1)))   # [128, 32]
    c["ident"] = np.eye(128, dtype=np.float32)
    return c


def _ids_for_core(inputs_np, core, T_):
    ids_c = inputs_np[core * BL:(core + 1) * BL, :T_]            # [16, T]
    flat = np.ascontiguousarray(ids_c.T).reshape(-1)             # t-major
    nblk = (BL * T_) // 128
    return np.ascontiguousarray(flat.reshape(nblk, 128).T.astype(np.int32))


def _build_program(T_):
    """Build the full single-core Bass program (identical across cores)."""
    TOK = BL * T_
    NBLK = TOK // 128

    nc = bacc.Bacc()
    d = {}

    def din(name, shape, dtype=f32):
        d[name] = nc.dram_tensor(name, list(shape), dtype, kind="ExternalInput")
        return d[name]

    din("ids_p", [128, NBLK], i32)
    din("embt", [V, D])
    for dd in (0, 1):
        din(f"wx0{dd}", [128, 512])
        din(f"wh0{dd}", [128, 512])
        din(f"wxA1{dd}", [128, 512])
        din(f"wxB1{dd}", [128, 512])
        din(f"wh1{dd}", [128, 512])
    din("biasK0", [8, 128])
    din("biasK1", [8, 128])
    din("chunkInd8", [8, 128])
    din("woutA", [128, K])
    din("woutB", [128, K])
    din("bo1", [1, K])
    din("ones1", [1, 128])
    for q in range(8):
        din(f"band{q}", [128, 128])
    din("transP", [128, 128])
    din("iotaI", [128, 128])
    din("iotaJP", [BL, K])
    din("iotaF", [BL, K])
    din("ms0", [128, 4])
    din("end128", [128, K])
    din("ident", [128, 128])
    out_ids = nc.dram_tensor("out_ids", [BL, T_], i32, kind="ExternalOutput")

    with tile.TileContext(nc) as tc:
        _emit(nc, tc, d, out_ids, T_, TOK, NBLK)
    nc.compile()
    return nc


def _lstm_layer(nc, wk, zpool, T_, xparts_f, xparts_b, wh_f, wh_b,
                biasK_sb, chunkInd_sb, hF, hB):
    """One BiLSTM layer; fwd and bwd directions interleaved per step.

    xparts_*: list of (lhsT_sbuf [128,512], rhs_fn(t) -> AP[128,16]) K-parts.
    hF/hB: [128, 16*T_] output buffers (also read for the recurrent matmul).
    """
    cF = wk.tile([128, 16], f32, tag="cF")
    cB = wk.tile([128, 16], f32, tag="cB")
    cs = (cF, cB)

    for s in range(T_):
        tf, tb = s, T_ - 1 - s
        zp = zpool.tile([128, 128], f32, tag="z")
        nc.tensor.matmul(out=zp[:], lhsT=biasK_sb[:], rhs=chunkInd_sb[:],
                         start=True, stop=False, skip_group_check=True)
        for dd, t, xparts, wh, hprev in (
            (0, tf, xparts_f, wh_f, hF),
            (1, tb, xparts_b, wh_b, hB),
        ):
            for ci in range(4):
                oap = zp[:, dd * 64 + ci * 16: dd * 64 + (ci + 1) * 16]
                mms = [(wT[:, ci * 128:(ci + 1) * 128], rhs_fn(t))
                       for (wT, rhs_fn) in xparts]
                if s > 0:
                    tp = t - 1 if dd == 0 else t + 1
                    mms.append((wh[:, ci * 128:(ci + 1) * 128],
                                hprev[:, 16 * tp:16 * tp + 16]))
                for j, (lh, rh) in enumerate(mms):
                    nc.tensor.matmul(out=oap, lhsT=lh, rhs=rh,
                                     start=False, stop=(j == len(mms) - 1),
                                     skip_group_check=True)

        # fwd and bwd get fully separate ACT/DVE chains so the scheduler can
        # pipeline one direction's gates against the other's matmuls.
        zp3 = zp.rearrange("p (d c2) -> p d c2", d=2)
        for dd, t, hout in ((0, tf, hF), (1, tb, hB)):
            c_d = cs[dd]
            sig = wk.tile([128, 48], f32, tag=f"sig{dd}",
                          name=f"sig{dd}_{s}")
            nc.scalar.activation(out=sig[:], in_=zp3[:, dd, 0:48],
                                 func=AF.Sigmoid)
            tg = wk.tile([128, 16], f32, tag=f"tg{dd}", name=f"tg{dd}_{s}")
            nc.scalar.activation(out=tg[:], in_=zp3[:, dd, 48:64],
                                 func=AF.Tanh)
            if s == 0:
                nc.vector.tensor_mul(out=c_d[:], in0=sig[:, 0:16], in1=tg[:])
            else:
                t1 = wk.tile([128, 16], f32, tag=f"t1{dd}",
                             name=f"t1{dd}_{s}")
                nc.vector.tensor_mul(out=t1[:], in0=sig[:, 0:16], in1=tg[:])
                t2 = wk.tile([128, 16], f32, tag=f"t2{dd}",
                             name=f"t2{dd}_{s}")
                nc.vector.tensor_mul(out=t2[:], in0=sig[:, 16:32],
                                     in1=c_d[:])
                nc.vector.tensor_add(out=c_d[:], in0=t1[:], in1=t2[:])
            tct = wk.tile([128, 16], f32, tag=f"tct{dd}",
                          name=f"tct{dd}_{s}")
            nc.scalar.activation(out=tct[:], in_=c_d[:], func=AF.Tanh)
            nc.vector.tensor_mul(out=hout[:, 16 * t:16 * t + 16],
                                 in0=sig[:, 32:48], in1=tct[:])


def _emit(nc, tc, d, out_ids, T_, TOK, NBLK):
    # bisection aid: stop after a given stage (embed, l0, l1, proj, vit, full)
    stage = os.environ.get("KERNEL_STAGE", "full")
    order = ["embed", "l0", "l1", "proj", "vit", "full"]
    lvl = order.index(stage)
    with tc.tile_pool(name="gc", bufs=1) as gc:
        band_sb = [gc.tile_from(d[f"band{q}"][:], name=f"band{q}sb")
                   for q in range(8)]
        transP_sb = gc.tile_from(d["transP"][:])
        iotaI_sb = gc.tile_from(d["iotaI"][:])
        iotaJP_sb = gc.tile_from(d["iotaJP"][:])
        iotaF_sb = gc.tile_from(d["iotaF"][:])
        ms0_sb = gc.tile_from(d["ms0"][:])
        end_sb = gc.tile_from(d["end128"][:])
        ident_sb = gc.tile_from(d["ident"][:])
        ids_sb = gc.tile_from(d["ids_p"][:])
        woutA_sb = gc.tile_from(d["woutA"][:])
        woutB_sb = gc.tile_from(d["woutB"][:])
        bo1_sb = gc.tile_from(d["bo1"][:])
        ones1_sb = gc.tile_from(d["ones1"][:])

        outT = gc.tile([BL, T_], f32)
        em2 = gc.tile([128, NBLK * K], f32)
        histAll = gc.tile([128, 4 * T_], f32)
        out_sb = gc.tile([BL, T_], i32)

        transP3 = transP_sb.rearrange("p (g i) -> p g i", g=4)
        iotaI3 = iotaI_sb.rearrange("p (g i) -> p g i", g=4)

        with tc.tile_pool(name="h1p", bufs=1) as h1p:
            h1F = h1p.tile([128, TOK], f32)
            h1B = h1p.tile([128, TOK], f32)

            # ---------------- embedding gather + layer 0 ----------------
            with tc.tile_pool(name="px", bufs=1) as px, \
                 tc.tile_pool(name="ge", bufs=4) as ge, \
                 tc.tile_pool(name="pe", bufs=2, space="PSUM") as pe, \
                 tc.tile_pool(name="zp0", bufs=2, space="PSUM") as zp0, \
                 tc.tile_pool(name="wk0", bufs=3) as wk0:
                xT = px.tile([128, TOK], f32)
                w0 = {dd: (px.tile_from(d[f"wx0{dd}"][:], name=f"wx0{dd}sb"),
                           px.tile_from(d[f"wh0{dd}"][:], name=f"wh0{dd}sb"))
                      for dd in (0, 1)}
                biasK0_sb = px.tile_from(d["biasK0"][:])
                chunkInd_sb = px.tile_from(d["chunkInd8"][:])

                # gather order: both ends first so fwd/bwd can start early
                order = []
                for k in range((NBLK + 1) // 2):
                    order.append(k)
                    if NBLK - 1 - k != k:
                        order.append(NBLK - 1 - k)
                for n, k in enumerate(order):
                    g_t = ge.tile([128, 128], f32, tag="g")
                    nc.gpsimd.indirect_dma_start(
                        out=g_t[:], out_offset=None, in_=d["embt"][:],
                        in_offset=IndirectOffsetOnAxis(
                            ap=ids_sb[:, k:k + 1], axis=0))
                    tp = pe.tile([128, 128], f32, tag="tp")
                    nc.tensor.transpose(tp[:], g_t[:], ident_sb[:])
                    dst = xT[:, 128 * k:128 * (k + 1)]
                    if n % 2 == 0:
                        nc.vector.tensor_copy(out=dst, in_=tp[:])
                    else:
                        nc.scalar.copy(out=dst, in_=tp[:])

                xf = [(w0[0][0], lambda t: xT[:, 16 * t:16 * t + 16])]
                xb = [(w0[1][0], lambda t: xT[:, 16 * t:16 * t + 16])]
                if lvl >= 1:
                    _lstm_layer(nc, wk0, zp0, T_, xf, xb, w0[0][1], w0[1][1],
                                biasK0_sb, chunkInd_sb, h1F, h1B)

            # ---------------- layer 1 + emission projection ----------------
            with tc.tile_pool(name="pw1", bufs=1) as pw1:
                w1 = {dd: (pw1.tile_from(d[f"wxA1{dd}"][:], name=f"wxA1{dd}sb"),
                           pw1.tile_from(d[f"wxB1{dd}"][:], name=f"wxB1{dd}sb"),
                           pw1.tile_from(d[f"wh1{dd}"][:], name=f"wh1{dd}sb"))
                      for dd in (0, 1)}
                biasK1_sb = pw1.tile_from(d["biasK1"][:])
                chunkInd1_sb = pw1.tile_from(d["chunkInd8"][:])

                with tc.tile_pool(name="ph2", bufs=1) as ph2, \
                     tc.tile_pool(name="zp1", bufs=2, space="PSUM") as zp1, \
                     tc.tile_pool(name="wk1", bufs=3) as wk1, \
                     tc.tile_pool(name="pj", bufs=2, space="PSUM") as pj:
                    h2F = ph2.tile([128, TOK], f32)
                    h2B = ph2.tile([128, TOK], f32)

                    xf = [(w1[0][0], lambda t: h1F[:, 16 * t:16 * t + 16]),
                          (w1[0][1], lambda t: h1B[:, 16 * t:16 * t + 16])]
                    xb = [(w1[1][0], lambda t: h1F[:, 16 * t:16 * t + 16]),
                          (w1[1][1], lambda t: h1B[:, 16 * t:16 * t + 16])]
                    if lvl >= 2:
                        _lstm_layer(nc, wk1, zp1, T_, xf, xb, w1[0][2],
                                    w1[1][2], biasK1_sb, chunkInd1_sb,
                                    h2F, h2B)

                    # emissions: em2[(t%8)*16+b, (t//8)*32+j]
                    for k in range(NBLK if lvl >= 3 else 0):
                        ep = pj.tile([128, K], f32, tag="ep")
                        nc.tensor.matmul(out=ep[:], lhsT=ones1_sb[:],
                                         rhs=bo1_sb[:], start=True, stop=False,
                                         skip_group_check=True)
                        nc.tensor.matmul(out=ep[:],
                                         lhsT=h2F[:, 128 * k:128 * (k + 1)],
                                         rhs=woutA_sb[:], start=False,
                                         stop=False, skip_group_check=True)
                        nc.tensor.matmul(out=ep[:],
                                         lhsT=h2B[:, 128 * k:128 * (k + 1)],
                                         rhs=woutB_sb[:], start=False,
                                         stop=True, skip_group_check=True)
                        dst = em2[:, K * k:K * (k + 1)]
                        if k % 2 == 0:
                            nc.vector.tensor_copy(out=dst, in_=ep[:])
                        else:
                            nc.scalar.copy(out=dst, in_=ep[:])

        # ---------------- Viterbi forward ----------------
        hA3 = histAll.rearrange("p (g t) -> p g t", t=T_)
        with tc.tile_pool(name="srp", bufs=2, space="PSUM") as srp, \
             tc.tile_pool(name="vt", bufs=3) as vt:
            nc.vector.memset(hA3[:, :, 0], 0.0)
            prev_ms = ms0_sb[:]
            for s in range(1, (T_ + 1) if lvl >= 4 else 0):
                sr = srp.tile([128, K], f32, tag="sr")
                srg = sr.rearrange("p (jl g) -> p jl g", g=4)
                tl = (s - 1) % 8
                blk = (s - 1) // 8
                nc.tensor.matmul(out=sr[:],
                                 lhsT=band_sb[tl][:],
                                 rhs=em2[:, K * blk:K * (blk + 1)],
                                 start=True, stop=False,
                                 skip_group_check=True)
                for jl in range(8):
                    nc.tensor.matmul(out=srg[:, jl, :],
                                     lhsT=band_sb[jl][:],
                                     rhs=prev_ms,
                                     start=False,
                                     stop=(jl == 7 and s != T_),
                                     skip_group_check=True)
                if s == T_:
                    # fold end transition scores into the final step
                    nc.tensor.matmul(out=sr[:], lhsT=band_sb[0][:],
                                     rhs=end_sb[:], start=False,
                                     stop=True, skip_group_check=True)
                    sf = vt.tile([BL, K], f32, tag="sf")
                    nc.vector.tensor_copy(out=sf[:], in_=sr[0:16, :])
                    mfin = vt.tile([BL, 1], f32, tag="mfin")
                    nc.vector.reduce_max(out=mfin[:], in_=sf[:], axis=AX.X)
                    eqf = vt.tile([BL, K], f32, tag="eqf")
                    nc.vector.tensor_tensor(
                        out=eqf[:], in0=sf[:],
                        in1=mfin[:].to_broadcast([BL, K]), op=ALU.is_equal)
                    eif = vt.tile([BL, K], f32, tag="eif")
                    nc.vector.tensor_mul(out=eif[:], in0=eqf[:],
                                         in1=iotaF_sb[:])
                    nc.vector.reduce_max(out=outT[:, T_ - 1:T_], in_=eif[:],
                                         axis=AX.X)
                    break

                cand = vt.tile([128, 128], f32, tag="cand")
                cand3 = cand.rearrange("p (g i) -> p g i", g=4)
                srb = sr[:].rearrange("p (o i) -> p o i", o=1) \
                           .to_broadcast([128, 4, K])
                nc.vector.tensor_add(out=cand3, in0=srb, in1=transP3)
                ms = vt.tile([128, 4], f32, tag="ms")
                nc.vector.reduce_max(out=ms[:], in_=cand3, axis=AX.X)
                eqv = vt.tile([128, 128], f32, tag="eqv")
                eq3 = eqv.rearrange("p (g i) -> p g i", g=4)
                msb = ms[:].rearrange("p (g o) -> p g o", o=1) \
                           .to_broadcast([128, 4, K])
                nc.vector.tensor_tensor(out=eq3, in0=cand3, in1=msb,
                                        op=ALU.is_equal)
                eiv = vt.tile([128, 128], f32, tag="eiv")
                ei3 = eiv.rearrange("p (g i) -> p g i", g=4)
                nc.vector.tensor_mul(out=ei3, in0=eq3, in1=iotaI3)
                nc.vector.reduce_max(out=hA3[:, :, s], in_=ei3, axis=AX.X)
                prev_ms = ms[:]

        # ---------------- backtrace ----------------
        with tc.tile_pool(name="pbt", bufs=1) as pbt, \
             tc.tile_pool(name="bt", bufs=2) as bt:
            if lvl >= 5:
                histAllB = pbt.tile([128, 4 * T_], bf16)
                nc.vector.tensor_copy(out=histAllB[:], in_=histAll[:])
                histB = pbt.tile([BL, 32 * T_], bf16)
                hB4 = histB.rearrange("p (jl g t) -> p jl g t", jl=8, g=4)
                for jl in range(8):
                    src = histAllB[16 * jl:16 * (jl + 1), :] \
                        .rearrange("p (g t) -> p g t", t=T_)
                    nc.sync.dma_start(out=hB4[:, jl], in_=src)

                for s in range(T_ - 2, -1, -1):
                    oh = bt.tile([BL, K], bf16, tag="oh")
                    nc.vector.tensor_scalar(out=oh[:], in0=iotaJP_sb[:],
                                            scalar1=outT[:, s + 1:s + 2],
                                            scalar2=None, op0=ALU.is_equal)
                    oh3 = oh.rearrange("p (jl g) -> p jl g", jl=8)
                    scr = bt.tile([BL, K], bf16, tag="scr")
                    scr3 = scr.rearrange("p (jl g) -> p jl g", jl=8)
                    nc.vector.tensor_mul(out=scr3, in0=oh3,
                                         in1=hB4[:, :, :, s + 1])
                    nc.vector.reduce_max(out=outT[:, s:s + 1], in_=scr[:],
                                         axis=AX.X)

                nc.vector.tensor_copy(out=out_sb[:], in_=outT[:])
            else:
                nc.vector.memset(out_sb[:], 0)
            nc.sync.dma_start(out=out_ids[:], in_=out_sb[:])


def _run(inputs_np, consts, T_):
    global LAST_RESULTS
    nc = _build_program(T_)
    in_maps = []
    for core in range(NCORES):
        m = dict(consts)
        m["ids_p"] = _ids_for_core(inputs_np, core, T_)
        in_maps.append(m)
    trace = bool(int(os.environ.get("KERNEL_TRACE", "0")))
    res = bass_utils.run_bass_kernel_spmd(
        nc, in_maps, core_ids=list(range(NCORES)), trace=trace)
    LAST_RESULTS = res
    return np.concatenate([r["out_ids"] for r in res.results], axis=0)


def kernel(inputs, tags, emb, w_ih_l0, w_hh_l0, b_l0,
           w_ih_l1, w_hh_l1, b_l1, W_out, b_out,
           start_t, end_t, trans, _T=TFULL):
    del tags  # unused at decode time
    inputs_np = np.ascontiguousarray(np.asarray(inputs, dtype=np.int32))
    consts = _host_consts(emb, w_ih_l0, w_hh_l0, b_l0, w_ih_l1, w_hh_l1,
                          b_l1, W_out, b_out, start_t, end_t, trans)
    return _run(inputs_np, consts, _T)



# revision 10
# speedup vs baseline: 2.3009x; 2.3009x over previous
"""BiLSTM-CRF decode kernel for Trainium2 (8 NeuronCores, batch-sharded).

Model: embedding lookup -> 2-layer BiLSTM (H=128/dir) -> linear -> CRF Viterbi.
Output: [B, T] int32 best-path tags.

v2: all matmuls run as manual hi/lo bf16 decompositions (3 bf16 matmuls
reproduce fp32-level precision at 1 cycle/row + fast weight load, vs the
hardware fp32 LOW_HIGH path at 4 cycles/row). The input-projection matmuls
are hoisted out of the serial recurrence into bulk weight-stationary
matmuls staged per TC-step chunk; biases are folded in during the
PSUM->SBUF copies. Viterbi keeps fp32 scores on the DVE; score/emission
values enter the PE score-replication matmuls via 3-way bf16 splits.
"""

import os
import numpy as np
import ml_dtypes

import concourse.bass as bass
import concourse.bacc as bacc
import concourse.tile as tile
import concourse.mybir as mybir
from concourse.bass import IndirectOffsetOnAxis
from concourse import bass_utils

B, TFULL, V, D, H, K = 128, 512, 50000, 128, 128, 32
NCORES = 8
BL = B // NCORES  # 16 batch rows per core
TC = 16           # recurrence steps staged per xpt chunk

f32 = mybir.dt.float32
bf16 = mybir.dt.bfloat16
i32 = mybir.dt.int32
AF = mybir.ActivationFunctionType
ALU = mybir.AluOpType
AX = mybir.AxisListType
NPBF = ml_dtypes.bfloat16

# torch gate order is [i, f, g, o]; we use [i, f, o, g] so the sigmoid gates
# (i, f, o) are contiguous and tanh(g) is the last chunk.
_PERM = np.r_[0:H, H:2 * H, 3 * H:4 * H, 2 * H:3 * H]

LAST_RESULTS = None  # BassKernelResults of the most recent run (for test.py)


def _f(x):
    return np.ascontiguousarray(np.asarray(x, dtype=np.float32))


def _b(x):
    return np.ascontiguousarray(np.asarray(x, dtype=np.float32).astype(NPBF))


def _split2(x):
    x = np.asarray(x, dtype=np.float32)
    hi = x.astype(NPBF)
    lo = (x - hi.astype(np.float32)).astype(NPBF)
    return np.ascontiguousarray(hi), np.ascontiguousarray(lo)


def _split3(x):
    x = np.asarray(x, dtype=np.float32)
    hi = x.astype(NPBF)
    r = x - hi.astype(np.float32)
    mid = r.astype(NPBF)
    lo = (r - mid.astype(np.float32)).astype(NPBF)
    return (np.ascontiguousarray(hi), np.ascontiguousarray(mid),
            np.ascontiguousarray(lo))


def _host_consts(emb, w_ih_l0, w_hh_l0, b_l0, w_ih_l1, w_hh_l1, b_l1,
                 W_out, b_out, start_t, end_t, trans):
    """Build all per-core-identical device input arrays."""
    c = {}
    c["embt"] = _f(emb)

    for d in (0, 1):
        for nm, w in (("wx0", np.asarray(w_ih_l0)[d][_PERM].T),
                      ("wh0", np.asarray(w_hh_l0)[d][_PERM].T),
                      ("wh1", np.asarray(w_hh_l1)[d][_PERM].T)):
            hi, lo = _split2(w)                              # [128, 512]
            c[f"{nm}hi{d}"], c[f"{nm}lo{d}"] = hi, lo
        w1 = np.asarray(w_ih_l1)[d][_PERM]                   # [512, 256]
        for nm, w in (("wxA1", w1[:, :H].T), ("wxB1", w1[:, H:].T)):
            hi, lo = _split2(w)
            c[f"{nm}hi{d}"], c[f"{nm}lo{d}"] = hi, lo

    for l, bl in ((0, b_l0), (1, b_l1)):
        bt = np.zeros((128, 8), dtype=np.float32)
        for d in (0, 1):
            bp = np.asarray(bl)[d][_PERM]
            for ci in range(4):
                bt[:, d * 4 + ci] = bp[ci * 128:(ci + 1) * 128]
        c[f"biasT{l}"] = _f(bt)

    c["identB"] = _b(np.eye(128, dtype=np.float32))
    c["identF"] = np.eye(128, dtype=np.float32)
    c["ones1"] = _b(np.ones((1, 128), dtype=np.float32))

    # Viterbi score columns are stored jl-major: column i' holds tag
    # pi[i'] = (i' % 4) * 8 + i' // 4.
    pi = (np.arange(32) % 4) * 8 + np.arange(32) // 4

    WoT = _f(np.asarray(W_out).T)                            # [256, 32]
    for nm, w in (("woutA", WoT[:H][:, pi]), ("woutB", WoT[H:][:, pi])):
        hi, lo = _split2(w)
        c[f"{nm}hi"], c[f"{nm}lo"] = hi, lo
    (c["bo1hi"], c["bo1mid"], c["bo1lo"]) = _split3(
        np.asarray(b_out)[pi][None, :])                      # [1, 32]

    km = np.arange(128)
    rep_full = (km[:, None] % 16 == km[None, :] % 16).astype(np.float32)
    for q in range(8):
        bm = rep_full.copy()
        bm[(km // 16) != q, :] = 0.0
        c[f"band{q}"] = _b(bm)

    jlv = np.arange(128) // 16                               # [128]
    gv = 8 * np.arange(4)                                    # [4]
    trans_np = _f(trans)
    # transP[p, g*32 + i'] = trans[pi[i'], g*8 + jl(p)]
    c["transP"] = _f(trans_np[pi].T[jlv[:, None] + gv[None, :]]
                     .reshape(128, 128))
    c["iotaI"] = _f(np.tile(pi.astype(np.float32), (128, 4)))
    c["iotaF"] = _f(np.tile(pi.astype(np.float32), (BL, 1)))  # [16, 32]

    jp = np.empty(32, dtype=np.float32)
    for jl in range(8):
        for g in range(4):
            jp[jl * 4 + g] = g * 8 + jl
    c["iotaJP"] = _f(np.tile(jp, (BL, 1)))                   # [16, 32]

    ms0 = np.asarray(start_t)[jlv[:, None] + gv[None, :]]    # [128, 4]
    c["ms0hi"], c["ms0mid"], c["ms0lo"] = _split3(ms0)
    end128 = np.tile(np.asarray(end_t)[pi], (128, 1))        # [128, 32]
    c["endhi"], c["endmid"], c["endlo"] = _split3(end128)
    return c


def _ids_for_core(inputs_np, core, T_):
    ids_c = inputs_np[core * BL:(core + 1) * BL, :T_]        # [16, T]
    flat = np.ascontiguousarray(ids_c.T).reshape(-1)         # t-major
    nblk = (BL * T_) // 128
    return np.ascontiguousarray(flat.reshape(nblk, 128).T.astype(np.int32))


def _build_program(T_):
    TOK = BL * T_
    NBLK = TOK // 128

    nc = bacc.Bacc()
    d = {}

    def din(name, shape, dtype=f32):
        d[name] = nc.dram_tensor(name, list(shape), dtype, kind="ExternalInput")
        return d[name]

    din("ids_p", [128, NBLK], i32)
    din("embt", [V, D])
    for dd in (0, 1):
        for nm in ("wx0", "wh0", "wxA1", "wxB1", "wh1"):
            din(f"{nm}hi{dd}", [128, 512], bf16)
            din(f"{nm}lo{dd}", [128, 512], bf16)
    din("biasT0", [128, 8])
    din("biasT1", [128, 8])
    din("identB", [128, 128], bf16)
    din("identF", [128, 128])
    din("ones1", [1, 128], bf16)
    for nm in ("woutAhi", "woutAlo", "woutBhi", "woutBlo"):
        din(nm, [128, K], bf16)
    for nm in ("bo1hi", "bo1mid", "bo1lo"):
        din(nm, [1, K], bf16)
    for q in range(8):
        din(f"band{q}", [128, 128], bf16)
    din("transP", [128, 128])
    din("iotaI", [128, 128])
    din("iotaJP", [BL, K])
    din("iotaF", [BL, K])
    for nm in ("ms0hi", "ms0mid", "ms0lo"):
        din(nm, [128, 4], bf16)
    for nm in ("endhi", "endmid", "endlo"):
        din(nm, [128, K], bf16)
    out_ids = nc.dram_tensor("out_ids", [BL, T_], i32, kind="ExternalOutput")
    if os.environ.get("KERNEL_DEBUG", "0") != "0":
        for nm in ("dbg_h1f", "dbg_h1b", "dbg_h2f", "dbg_h2b"):
            d[nm] = nc.dram_tensor(nm, [128, TOK], f32, kind="ExternalOutput")
        d["dbg_em"] = nc.dram_tensor("dbg_em", [128, NBLK * K], f32,
                                     kind="ExternalOutput")
        d["dbg_z1"] = nc.dram_tensor("dbg_z1", [128, 128], f32,
                                     kind="ExternalOutput")

    with tile.TileContext(nc) as tc:
        _emit(nc, tc, d, out_ids, T_, TOK, NBLK)
    nc.compile()
    return nc


def _emit_bulk_chunk(nc, xpp, bulkp, parts, biasT_sb, c, T_, name):
    """Bulk input-projection for one (chunk, both dirs): returns xpt tiles.

    parts[d] = list of (whi_sb, wlo_sb, srchi, srclo) contraction groups.
    Output: dict (d, 'hi'|'lo') -> [128, TC*64] bf16 tile laid out
    [p, (tloc, ci, b)] with bias folded in.
    """
    out = {}
    for dd in (0, 1):
        t0 = c * TC if dd == 0 else T_ - (c + 1) * TC
        xhi = xpp.tile([128, TC * 64], bf16, tag=f"x{dd}hi",
                       name=f"x{dd}hi_{name}_{c}")
        xlo = xpp.tile([128, TC * 64], bf16, tag=f"x{dd}lo",
                       name=f"x{dd}lo_{name}_{c}")
        xhi4 = xhi.rearrange("p (t c2 b) -> p t c2 b", t=TC, c2=4)
        xlo4 = xlo.rearrange("p (t c2 b) -> p t c2 b", t=TC, c2=4)
        for ci in range(4):
            ps = bulkp.tile([128, TC * 16], f32, tag="bp",
                            name=f"bp_{name}_{c}_{dd}_{ci}")
            nmm = 3 * len(parts[dd])
            j = 0
            for (whi, wlo, shi, slo) in parts[dd]:
                rhs_hi = shi[:, 16 * t0:16 * (t0 + TC)]
                rhs_lo = slo[:, 16 * t0:16 * (t0 + TC)]
                lh_hi = whi[:, ci * 128:(ci + 1) * 128]
                lh_lo = wlo[:, ci * 128:(ci + 1) * 128]
                for lh, rh in ((lh_hi, rhs_hi), (lh_hi, rhs_lo),
                               (lh_lo, rhs_hi)):
                    nc.tensor.matmul(out=ps[:], lhsT=lh, rhs=rh,
                                     start=(j == 0), stop=(j == nmm - 1),
                                     skip_group_check=True)
                    j += 1
            ps3 = ps.rearrange("p (t b) -> p t b", b=16)
            bias = biasT_sb[:, dd * 4 + ci:dd * 4 + ci + 1]
            # hi = bf16(psum + bias) on ACT; lo = (psum + bias) - hi on DVE
            nc.scalar.add(out=xhi4[:, :, ci, :], in_=ps3, add=bias)
            nc.vector.scalar_tensor_tensor(
                out=xlo4[:, :, ci, :], in0=ps3, scalar=bias,
                in1=xhi4[:, :, ci, :], op0=ALU.add, op1=ALU.subtract)
        out[(dd, "hi")] = xhi
        out[(dd, "lo")] = xlo
    return out


def _emit_layer(nc, tc, d, T_, parts, whh, biasT_sb, identB_sb, hout,
                gather_fn, name):
    """One BiLSTM layer: staged bulk x-projection + serial recurrence.

    whh[d] = (whhhi_sb, whhlo_sb); hout[(d, 'hi'|'lo')] = [128, TOK] bf16.
    gather_fn(c): emit embedding gathers needed by bulk chunk c (L0 only).
    """
    NCH = T_ // TC
    with tc.tile_pool(name=f"xpp{name}", bufs=2) as xpp, \
         tc.tile_pool(name=f"bulkp{name}", bufs=2, space="PSUM") as bulkp, \
         tc.tile_pool(name=f"zp{name}", bufs=3, space="PSUM") as zpool, \
         tc.tile_pool(name=f"wk{name}", bufs=3) as wk, \
         tc.tile_pool(name=f"cp{name}", bufs=1) as cp:
        cF = cp.tile([128, 16], f32, name=f"cF{name}")
        cB = cp.tile([128, 16], f32, name=f"cB{name}")
        cs = (cF, cB)

        if gather_fn is not None:
            gather_fn(0)
        tiles = {0: _emit_bulk_chunk(nc, xpp, bulkp, parts, biasT_sb, 0,
                                     T_, name)}
        for c in range(NCH):
            if c + 1 < NCH:
                if gather_fn is not None:
                    gather_fn(c + 1)
                tiles[c + 1] = _emit_bulk_chunk(nc, xpp, bulkp, parts,
                                                biasT_sb, c + 1, T_, name)
            xt = tiles.pop(c)
            for sloc in range(TC):
                s = c * TC + sloc
                tf, tb = s, T_ - 1 - s
                zp = zpool.tile([128, 128], f32, tag="z", name=f"z{name}_{s}")
                # ident-fold of the precomputed x-part (+bias) into PSUM
                # start=True only on the very first matmul touching this zp:
                # start clears has_written bank-wide, so a second start=True
                # would wipe the other half's accumulation state.
                for dd in (0, 1):
                    tloc = sloc if dd == 0 else TC - 1 - sloc
                    oap = zp[:, dd * 64:(dd + 1) * 64]
                    for j, part in enumerate(("hi", "lo")):
                        rhs = xt[(dd, part)][:, tloc * 64:(tloc + 1) * 64]
                        nc.tensor.matmul(out=oap, lhsT=identB_sb[:], rhs=rhs,
                                         start=(dd == 0 and j == 0),
                                         stop=(dd == 1 and j == 1 and s == 0),
                                         skip_group_check=True)
                if s > 0:
                    for dd, t in ((0, tf), (1, tb)):
                        tp = t - 1 if dd == 0 else t + 1
                        whhhi, whhlo = whh[dd]
                        rh_hi = hout[(dd, "hi")][:, 16 * tp:16 * tp + 16]
                        rh_lo = hout[(dd, "lo")][:, 16 * tp:16 * tp + 16]
                        for ci in range(4):
                            oap = zp[:, dd * 64 + ci * 16:dd * 64 + ci * 16 + 16]
                            lh_hi = whhhi[:, ci * 128:(ci + 1) * 128]
                            lh_lo = whhlo[:, ci * 128:(ci + 1) * 128]
                            for j, (lh, rh) in enumerate(
                                    ((lh_hi, rh_hi), (lh_hi, rh_lo),
                                     (lh_lo, rh_hi))):
                                nc.tensor.matmul(
                                    out=oap, lhsT=lh, rhs=rh, start=False,
                                    stop=(dd == 1 and ci == 3 and j == 2),
                                    skip_group_check=True)

                if (os.environ.get("KERNEL_DEBUG", "0") != "0" and s == 1
                        and name == "l0" and "dbg_z1" in d):
                    zc = cp.tile([128, 128], f32, name="zdbg")
                    nc.vector.tensor_copy(out=zc[:], in_=zp[:])
                    nc.sync.dma_start(out=d["dbg_z1"][:], in_=zc[:])

                zp3 = zp.rearrange("p (d c2) -> p d c2", d=2)
                # gate activations: all four z-dependent ACT ops first so the
                # two directions' chains pipeline without FIFO blocking
                sigs, tgs = [], []
                for dd in (0, 1):
                    sig = wk.tile([128, 48], f32, tag=f"sig{dd}",
                                  name=f"sig{dd}_{name}_{s}")
                    nc.scalar.activation(out=sig[:], in_=zp3[:, dd, 0:48],
                                         func=AF.Sigmoid)
                    tg = wk.tile([128, 16], f32, tag=f"tg{dd}",
                                 name=f"tg{dd}_{name}_{s}")
                    nc.scalar.activation(out=tg[:], in_=zp3[:, dd, 48:64],
                                         func=AF.Tanh)
                    sigs.append(sig)
                    tgs.append(tg)
                # cell update on DVE
                for dd in (0, 1):
                    sig, tg, c_d = sigs[dd], tgs[dd], cs[dd]
                    if s == 0:
                        nc.vector.tensor_mul(out=c_d[:], in0=sig[:, 0:16],
                                             in1=tg[:])
                    else:
                        t1 = wk.tile([128, 16], f32, tag=f"t1{dd}",
                                     name=f"t1{dd}_{name}_{s}")
                        nc.vector.tensor_mul(out=t1[:], in0=sig[:, 0:16],
                                             in1=tg[:])
                        t2 = wk.tile([128, 16], f32, tag=f"t2{dd}",
                                     name=f"t2{dd}_{name}_{s}")
                        nc.vector.tensor_mul(out=t2[:], in0=sig[:, 16:32],
                                             in1=c_d[:])
                        nc.vector.tensor_add(out=c_d[:], in0=t1[:], in1=t2[:])
                tcts = []
                for dd in (0, 1):
                    tct = wk.tile([128, 16], f32, tag=f"tct{dd}",
                                  name=f"tct{dd}_{name}_{s}")
                    nc.scalar.activation(out=tct[:], in_=cs[dd][:],
                                         func=AF.Tanh)
                    tcts.append(tct)
                for dd, t in ((0, tf), (1, tb)):
                    hf = wk.tile([128, 16], f32, tag=f"hf{dd}",
                                 name=f"hf{dd}_{name}_{s}")
                    nc.vector.tensor_mul(out=hf[:], in0=sigs[dd][:, 32:48],
                                         in1=tcts[dd][:])
                    dsthi = hout[(dd, "hi")][:, 16 * t:16 * t + 16]
                    nc.vector.tensor_copy(out=dsthi, in_=hf[:])
                    nc.vector.tensor_tensor(
                        out=hout[(dd, "lo")][:, 16 * t:16 * t + 16],
                        in0=hf[:], in1=dsthi, op=ALU.subtract)


def _emit(nc, tc, d, out_ids, T_, TOK, NBLK):
    # bisection aid: stop after a given stage (embed, l0, l1, em, vit, full)
    stage = os.environ.get("KERNEL_STAGE", "full")
    order = ["embed", "l0", "l1", "em", "vit", "full"]
    lvl = order.index(stage)
    NCH = T_ // TC
    BPC = NBLK // NCH  # embedding blocks per chunk per direction end

    with tc.tile_pool(name="gc", bufs=1) as gc:
        identB_sb = gc.tile_from(d["identB"][:])
        identF_sb = gc.tile_from(d["identF"][:])
        ids_sb = gc.tile_from(d["ids_p"][:])
        band_sb = [gc.tile_from(d[f"band{q}"][:], name=f"band{q}sb")
                   for q in range(8)]
        transP_sb = gc.tile_from(d["transP"][:])
        iotaI_sb = gc.tile_from(d["iotaI"][:])
        iotaJP_sb = gc.tile_from(d["iotaJP"][:])
        iotaF_sb = gc.tile_from(d["iotaF"][:])
        ms0_sb = tuple(gc.tile_from(d[nm][:], name=nm + "sb")
                       for nm in ("ms0hi", "ms0mid", "ms0lo"))
        end_sb = tuple(gc.tile_from(d[nm][:], name=nm + "sb")
                       for nm in ("endhi", "endmid", "endlo"))
        outT = gc.tile([BL, T_], f32)
        out_sb = gc.tile([BL, T_], i32)

        transP3 = transP_sb.rearrange("p (g i) -> p g i", g=4)
        iotaI3 = iotaI_sb.rearrange("p (g i) -> p g i", g=4)

        with tc.tile_pool(name="hp", bufs=1) as hp:
            h1 = {(dd, p): hp.tile([128, TOK], bf16, name=f"h1_{dd}{p}")
                  for dd in (0, 1) for p in ("hi", "lo")}
            h2 = {(dd, p): hp.tile([128, TOK], bf16, name=f"h2_{dd}{p}")
                  for dd in (0, 1) for p in ("hi", "lo")}

            # ---------------- embedding gather + layer 0 ----------------
            with tc.tile_pool(name="px", bufs=1) as px, \
                 tc.tile_pool(name="ge", bufs=4) as ge, \
                 tc.tile_pool(name="pe", bufs=2, space="PSUM") as pe:
                xThi = px.tile([128, TOK], bf16)
                xTlo = px.tile([128, TOK], bf16)
                w0 = {}
                for dd in (0, 1):
                    w0[dd] = tuple(
                        px.tile_from(d[f"{nm}{p}{dd}"][:],
                                     name=f"{nm}{p}{dd}sb")
                        for nm in ("wx0", "wh0") for p in ("hi", "lo"))
                biasT0_sb = px.tile_from(d["biasT0"][:])

                def gather_block(k):
                    g_t = ge.tile([128, 128], f32, tag="g", name=f"g_{k}")
                    nc.gpsimd.indirect_dma_start(
                        out=g_t[:], out_offset=None, in_=d["embt"][:],
                        in_offset=IndirectOffsetOnAxis(
                            ap=ids_sb[:, k:k + 1], axis=0))
                    tp = pe.tile([128, 128], f32, tag="tp", name=f"tp_{k}")
                    nc.tensor.transpose(tp[:], g_t[:], identF_sb[:])
                    dsthi = xThi[:, 128 * k:128 * (k + 1)]
                    nc.scalar.copy(out=dsthi, in_=tp[:])
                    nc.vector.tensor_tensor(
                        out=xTlo[:, 128 * k:128 * (k + 1)],
                        in0=tp[:], in1=dsthi, op=ALU.subtract)

                def gather_chunk(c):
                    if c >= NCH // 2:
                        return
                    for k in range(BPC * c, BPC * (c + 1)):
                        gather_block(k)
                        gather_block(NBLK - 1 - k)

                parts0 = {dd: [(w0[dd][0], w0[dd][1], xThi, xTlo)]
                          for dd in (0, 1)}
                whh0 = {dd: (w0[dd][2], w0[dd][3]) for dd in (0, 1)}
                if lvl >= 1:
                    _emit_layer(nc, tc, d, T_, parts0, whh0, biasT0_sb,
                                identB_sb, h1, gather_chunk, "l0")
                else:
                    for c in range(NCH):
                        gather_chunk(c)

            # ---------------- layer 1 ----------------
            with tc.tile_pool(name="pw1", bufs=1) as pw1:
                w1 = {}
                for dd in (0, 1):
                    w1[dd] = tuple(
                        pw1.tile_from(d[f"{nm}{p}{dd}"][:],
                                      name=f"{nm}{p}{dd}sb")
                        for nm in ("wxA1", "wxB1", "wh1")
                        for p in ("hi", "lo"))
                biasT1_sb = pw1.tile_from(d["biasT1"][:])
                parts1 = {dd: [(w1[dd][0], w1[dd][1], h1[(0, "hi")],
                                h1[(0, "lo")]),
                               (w1[dd][2], w1[dd][3], h1[(1, "hi")],
                                h1[(1, "lo")])]
                          for dd in (0, 1)}
                whh1 = {dd: (w1[dd][4], w1[dd][5]) for dd in (0, 1)}
                if lvl >= 2:
                    _emit_layer(nc, tc, d, T_, parts1, whh1, biasT1_sb,
                                identB_sb, h2, None, "l1")

            if os.environ.get("KERNEL_DEBUG", "0") != "0":
                with tc.tile_pool(name="dbg", bufs=2) as dbg:
                    for nm, ht in (("dbg_h1f", (h1, 0)), ("dbg_h1b", (h1, 1)),
                                   ("dbg_h2f", (h2, 0)), ("dbg_h2b", (h2, 1))):
                        hd, dd = ht
                        full = dbg.tile([128, TOK], f32, tag="full",
                                        name=nm + "t")
                        nc.vector.tensor_tensor(out=full[:],
                                                in0=hd[(dd, "hi")][:],
                                                in1=hd[(dd, "lo")][:],
                                                op=ALU.add)
                        nc.sync.dma_start(out=d[nm][:], in_=full[:])

            # ---------------- emissions + viterbi + backtrace ----------------
            with tc.tile_pool(name="emc", bufs=1) as emc, \
                 tc.tile_pool(name="emp", bufs=2, space="PSUM") as emp, \
                 tc.tile_pool(name="emw", bufs=3) as emw:
                woutAhi_sb = emc.tile_from(d["woutAhi"][:])
                woutAlo_sb = emc.tile_from(d["woutAlo"][:])
                woutBhi_sb = emc.tile_from(d["woutBhi"][:])
                woutBlo_sb = emc.tile_from(d["woutBlo"][:])
                ones1_sb = emc.tile_from(d["ones1"][:])
                bo_sb = tuple(emc.tile_from(d[nm][:], name=nm + "sb")
                              for nm in ("bo1hi", "bo1mid", "bo1lo"))
                em2 = {p: emc.tile([128, NBLK * K], bf16, name=f"em2{p}")
                       for p in ("hi", "mid", "lo")}
                histAll = emc.tile([128, 4 * T_], f32)
                hA3 = histAll.rearrange("p (g t) -> p g t", t=T_)

                # emissions: em2[(t%8)*16+b, (t//8)*32+j], 3-way bf16 split
                for k in range(NBLK if lvl >= 3 else 0):
                    ep = emp.tile([128, K], f32, tag="ep", name=f"ep_{k}")
                    mms = [(ones1_sb[:], bo_sb[0][:]),
                           (ones1_sb[:], bo_sb[1][:]),
                           (ones1_sb[:], bo_sb[2][:])]
                    blk = slice(128 * k, 128 * (k + 1))
                    mms += [(h2[(0, "hi")][:, blk], woutAhi_sb[:]),
                            (h2[(0, "hi")][:, blk], woutAlo_sb[:]),
                            (h2[(0, "lo")][:, blk], woutAhi_sb[:]),
                            (h2[(1, "hi")][:, blk], woutBhi_sb[:]),
                            (h2[(1, "hi")][:, blk], woutBlo_sb[:]),
                            (h2[(1, "lo")][:, blk], woutBhi_sb[:])]
                    for j, (lh, rh) in enumerate(mms):
                        nc.tensor.matmul(out=ep[:], lhsT=lh, rhs=rh,
                                         start=(j == 0),
                                         stop=(j == len(mms) - 1),
                                         skip_group_check=True)
                    esl = slice(K * k, K * (k + 1))
                    nc.scalar.copy(out=em2["hi"][:, esl], in_=ep[:])
                    rr = emw.tile([128, K], f32, tag="rr", name=f"rr_{k}")
                    nc.vector.tensor_tensor(out=rr[:], in0=ep[:],
                                            in1=em2["hi"][:, esl],
                                            op=ALU.subtract)
                    nc.gpsimd.tensor_copy(out=em2["mid"][:, esl], in_=rr[:])
                    nc.vector.tensor_tensor(out=em2["lo"][:, esl], in0=rr[:],
                                            in1=em2["mid"][:, esl],
                                            op=ALU.subtract)

                if os.environ.get("KERNEL_DEBUG", "0") != "0":
                    with tc.tile_pool(name="dbge", bufs=1) as dbge:
                        emf = dbge.tile([128, NBLK * K], f32)
                        nc.vector.tensor_tensor(out=emf[:], in0=em2["hi"][:],
                                                in1=em2["mid"][:], op=ALU.add)
                        nc.vector.tensor_tensor(out=emf[:], in0=emf[:],
                                                in1=em2["lo"][:], op=ALU.add)
                        nc.sync.dma_start(out=d["dbg_em"][:], in_=emf[:])

                # ---------------- Viterbi forward ----------------
                with tc.tile_pool(name="srp", bufs=2, space="PSUM") as srp, \
                     tc.tile_pool(name="vt", bufs=3) as vt:
                    nc.vector.memset(hA3[:, :, 0], 0.0)
                    prev = ms0_sb
                    pending_hist = None
                    for s in range(1, (T_ + 1) if lvl >= 4 else 0):
                        sr = srp.tile([128, K], f32, tag="sr",
                                      name=f"sr_{s}")
                        srg = sr.rearrange("p (jl g) -> p jl g", g=4)
                        tl = (s - 1) % 8
                        blk = (s - 1) // 8
                        for j, p in enumerate(("hi", "mid", "lo")):
                            nc.tensor.matmul(
                                out=sr[:], lhsT=band_sb[tl][:],
                                rhs=em2[p][:, K * blk:K * (blk + 1)],
                                start=(j == 0), stop=False,
                                skip_group_check=True)
                        for j in range(8):
                            jl = (tl + j) % 8
                            for pi_, part in enumerate(prev):
                                nc.tensor.matmul(
                                    out=srg[:, jl, :], lhsT=band_sb[jl][:],
                                    rhs=part[:], start=False,
                                    stop=(j == 7 and pi_ == 2 and s != T_),
                                    skip_group_check=True)
                        if s == T_:
                            for pi_, e in enumerate(end_sb):
                                nc.tensor.matmul(out=sr[:],
                                                 lhsT=band_sb[0][:],
                                                 rhs=e[:], start=False,
                                                 stop=(pi_ == 2),
                                                 skip_group_check=True)
                            sf = vt.tile([BL, K], f32, tag="sf")
                            nc.vector.tensor_copy(out=sf[:], in_=sr[0:16, :])
                            mfin = vt.tile([BL, 1], f32, tag="mfin")
                            nc.vector.reduce_max(out=mfin[:], in_=sf[:],
                                                 axis=AX.X)
                            eqf = vt.tile([BL, K], f32, tag="eqf")
                            nc.vector.tensor_tensor(
                                out=eqf[:], in0=sf[:],
                                in1=mfin[:].to_broadcast([BL, K]),
                                op=ALU.is_equal)
                            eif = vt.tile([BL, K], f32, tag="eif")
                            nc.vector.tensor_mul(out=eif[:], in0=eqf[:],
                                                 in1=iotaF_sb[:])
                            nc.vector.reduce_max(out=outT[:, T_ - 1:T_],
                                                 in_=eif[:], axis=AX.X)
                            break

                        cand = vt.tile([128, 128], f32, tag="cand",
                                       name=f"cand_{s}")
                        cand3 = cand.rearrange("p (g i) -> p g i", g=4)
                        srb = sr[:].rearrange("p (o i) -> p o i", o=1) \
                                   .to_broadcast([128, 4, K])
                        nc.vector.tensor_add(out=cand3, in0=srb, in1=transP3)
                        ms = vt.tile([128, 4], f32, tag="ms",
                                     name=f"ms_{s}")
                        nc.vector.reduce_max(out=ms[:], in_=cand3, axis=AX.X)
                        # 3-way bf16 split of the scores for the next flip
                        mhi = vt.tile([128, 4], bf16, tag="mhi",
                                      name=f"mhi_{s}")
                        nc.vector.tensor_copy(out=mhi[:], in_=ms[:])
                        rr = vt.tile([128, 4], f32, tag="rr", name=f"rrv_{s}")
                        nc.vector.tensor_tensor(out=rr[:], in0=ms[:],
                                                in1=mhi[:], op=ALU.subtract)
                        mmid = vt.tile([128, 4], bf16, tag="mmid",
                                       name=f"mmid_{s}")
                        nc.vector.tensor_copy(out=mmid[:], in_=rr[:])
                        mlo = vt.tile([128, 4], bf16, tag="mlo",
                                      name=f"mlo_{s}")
                        nc.vector.tensor_tensor(out=mlo[:], in0=rr[:],
                                                in1=mmid[:], op=ALU.subtract)
                        prev = (mhi, mmid, mlo)
                        # history (argmax) on gpsimd; final reduce on DVE,
                        # deferred one step to avoid FIFO head-of-line stalls
                        msb = ms[:].rearrange("p (g o) -> p g o", o=1) \
                                   .to_broadcast([128, 4, K])
                        dfv = vt.tile([128, 128], f32, tag="dfv",
                                      name=f"dfv_{s}")
                        df3 = dfv.rearrange("p (g i) -> p g i", g=4)
                        nc.gpsimd.tensor_tensor(out=df3, in0=cand3, in1=msb,
                                                op=ALU.subtract)
                        eqv = vt.tile([128, 128], f32, tag="eqv",
                                      name=f"eqv_{s}")
                        eq3 = eqv.rearrange("p (g i) -> p g i", g=4)
                        nc.gpsimd.tensor_scalar(out=eq3, in0=df3, scalar1=0.0,
                                                scalar2=None,
                                                op0=ALU.is_equal)
                        eiv = vt.tile([128, 128], f32, tag="eiv",
                                      name=f"eiv_{s}")
                        ei3 = eiv.rearrange("p (g i) -> p g i", g=4)
                        nc.gpsimd.tensor_mul(out=ei3, in0=eq3, in1=iotaI3)
                        if pending_hist is not None:
                            ph3, ps_ = pending_hist
                            nc.vector.reduce_max(out=hA3[:, :, ps_], in_=ph3,
                                                 axis=AX.X)
                        pending_hist = (ei3, s)
                    if pending_hist is not None:
                        ph3, ps_ = pending_hist
                        nc.vector.reduce_max(out=hA3[:, :, ps_], in_=ph3,
                                             axis=AX.X)

                # ---------------- backtrace ----------------
                with tc.tile_pool(name="pbt", bufs=1) as pbt, \
                     tc.tile_pool(name="bt", bufs=2) as bt:
                    if lvl >= 5:
                        histAllB = pbt.tile([128, 4 * T_], bf16)
                        nc.vector.tensor_copy(out=histAllB[:], in_=histAll[:])
                        histB = pbt.tile([BL, 32 * T_], bf16)
                        hB4 = histB.rearrange("p (jl g t) -> p jl g t",
                                              jl=8, g=4)
                        for jl in range(8):
                            src = histAllB[16 * jl:16 * (jl + 1), :] \
                                .rearrange("p (g t) -> p g t", t=T_)
                            nc.sync.dma_start(out=hB4[:, jl], in_=src)

                        iotaJP3 = iotaJP_sb.rearrange("p (jl g) -> p jl g",
                                                      jl=8)
                        for s in range(T_ - 2, -1, -1):
                            scr = bt.tile([BL, K], bf16, tag="scr",
                                          name=f"scr_{s}")
                            scr3 = scr.rearrange("p (jl g) -> p jl g", jl=8)
                            nc.vector.scalar_tensor_tensor(
                                out=scr3, in0=iotaJP3,
                                scalar=outT[:, s + 1:s + 2],
                                in1=hB4[:, :, :, s + 1],
                                op0=ALU.is_equal, op1=ALU.mult)
                            nc.vector.reduce_max(out=outT[:, s:s + 1],
                                                 in_=scr[:], axis=AX.X)

                        nc.vector.tensor_copy(out=out_sb[:], in_=outT[:])
                    else:
                        nc.vector.memset(out_sb[:], 0)
                    nc.sync.dma_start(out=out_ids[:], in_=out_sb[:])


def _run(inputs_np, consts, T_):
    global LAST_RESULTS
    nc = _build_program(T_)
    in_maps = []
    for core in range(NCORES):
        m = dict(consts)
        m["ids_p"] = _ids_for_core(inputs_np, core, T_)
        in_maps.append(m)
    trace = bool(int(os.environ.get("KERNEL_TRACE", "0")))
    res = bass_utils.run_bass_kernel_spmd(
        nc, in_maps, core_ids=list(range(NCORES)), trace=trace)
    LAST_RESULTS = res
    return np.concatenate([r["out_ids"] for r in res.results], axis=0)


def kernel(inputs, tags, emb, w_ih_l0, w_hh_l0, b_l0,
           w_ih_l1, w_hh_l1, b_l1, W_out, b_out,
           start_t, end_t, trans, _T=TFULL):
    del tags  # unused at decode time
    inputs_np = np.ascontiguousarray(np.asarray(inputs, dtype=np.int32))
    consts = _host_consts(emb, w_ih_l0, w_hh_l0, b_l0, w_ih_l1, w_hh_l1,
                          b_l1, W_out, b_out, start_t, end_t, trans)
    return _run(inputs_np, consts, _T)
